# revision 3
# baseline (speedup 1.0000x reference)
"""MiniCausalAttention on 8 NeuronCores (Trainium2, Bass/Tile).

Problem: x[4,2048,1024] fp32; q/k/v = x@w+b; causal softmax(q k^T/sqrt(D)) @ v.

Sharding: 8 cores = (batch b in 0..3) x (half h in 0..1). Core (b,h) handles
query tiles t' = 2t+h for t in 0..7 (interleaved 128-row tiles), so every
core sees the SAME set of causal key-extents nk(t) = 256*(t+1) -> one SPMD
program, perfectly balanced. Each core computes K^T and V for its batch's
full sequence (replicated within the pair), Q^T for its own 1024 rows.

All matmuls run in bf16 (full PE rate); PSUM accumulation is fp32; softmax
statistics fp32. Layouts: x^T/K^T/Q^T are [d_model partition, token free] so
scores S[q,k] = (Q^T)^T K^T comes out query-major; V is token-major so
O = P V after a PE transpose of each 128x128 P tile.
"""

import sys

if "/opt/trn_rl_repo" not in sys.path:
    sys.path.insert(0, "/opt/trn_rl_repo")

import numpy as np
import ml_dtypes

import concourse.bass as bass  # noqa: F401  (registers AP machinery)
import concourse.tile as tile
from concourse import bacc, mybir
from concourse.bass_utils import run_bass_kernel_spmd
from concourse.masks import make_identity

BF16 = mybir.dt.bfloat16
F32 = mybir.dt.float32
AF = mybir.ActivationFunctionType

B, L, D = 4, 2048, 1024
P = 128
NQT = 8          # q-tiles per core, 128 rows each
SCALE = 1.0 / 32.0   # 1/sqrt(D)
NEG = -1.0e30

_CACHED = {}


def build_nc():
    nc = bacc.Bacc(None, target_bir_lowering=False)

    xt = nc.declare_dram_parameter("xt", [D, L], BF16, isOutput=False)
    xtq = nc.declare_dram_parameter("xtq", [D, D], BF16, isOutput=False)
    wq = nc.declare_dram_parameter("wq", [D, D], BF16, isOutput=False)
    wk = nc.declare_dram_parameter("wk", [D, D], BF16, isOutput=False)
    wv = nc.declare_dram_parameter("wv", [D, D], BF16, isOutput=False)
    bqm = nc.declare_dram_parameter("bqm", [P, 8], F32, isOutput=False)
    bkm = nc.declare_dram_parameter("bkm", [P, 8], F32, isOutput=False)
    bvr = nc.declare_dram_parameter("bvr", [1, D], BF16, isOutput=False)
    mask = nc.declare_dram_parameter("mask", [P, 256], F32, isOutput=False)
    out = nc.declare_dram_parameter("out", [D, D], F32, isOutput=True)

    with tile.TileContext(nc) as tc:
        with tc.tile_pool(name="persist", bufs=1) as persist:
            kt_sb = persist.tile([P, 8, L], BF16)    # K^T: [d-part, dt, token]
            v_sb = persist.tile([P, 16, D], BF16)    # V: [tok-part, tt, d]
            qt_sb = persist.tile([P, 8, D], BF16)    # Q^T: [d-part, dt, qcol]
            bqm_sb = persist.tile([P, 8], F32)
            bkm_sb = persist.tile([P, 8], F32)
            bvr_sb = persist.tile([1, D], BF16)
            mask_sb = persist.tile([P, 256], F32)
            ident = persist.tile([P, P], BF16)
            ones_sb = persist.tile([1, P], BF16)

            nc.sync.dma_start(out=bqm_sb, in_=bqm[:, :])
            nc.sync.dma_start(out=bkm_sb, in_=bkm[:, :])
            nc.sync.dma_start(out=bvr_sb, in_=bvr[:, :])
            nc.sync.dma_start(out=mask_sb, in_=mask[:, :])
            make_identity(nc, ident)
            nc.vector.memset(ones_sb, 1.0)

            # ---------------- Phase A1/A2: V and K^T over full sequence ----
            with tc.tile_pool(name="a_in", bufs=1) as a_in:
                xt_sb = a_in.tile([P, 8, L], BF16)
                wk_sb = a_in.tile([P, 8, D], BF16)
                wv_sb = a_in.tile([P, 8, D], BF16)
                for i in range(8):
                    nc.sync.dma_start(out=xt_sb[:, i, :], in_=xt[i * P:(i + 1) * P, :])
                    nc.sync.dma_start(out=wk_sb[:, i, :], in_=wk[i * P:(i + 1) * P, :])
                    nc.sync.dma_start(out=wv_sb[:, i, :], in_=wv[i * P:(i + 1) * P, :])

                # A1: V[tok, d] = sum_ct xt[ct, tok].T @ wv[ct, d]
                with tc.tile_pool(name="psV", bufs=2, space="PSUM") as psV:
                    for tt in range(16):
                        pv = [psV.tile([P, 512], F32, tag=f"pv{dc}", name=f"pv{dc}") for dc in range(2)]
                        for ct in range(8):
                            for dc in range(2):
                                nc.tensor.matmul(
                                    pv[dc],
                                    xt_sb[:, ct, tt * P:(tt + 1) * P],
                                    wv_sb[:, ct, dc * 512:(dc + 1) * 512],
                                    start=(ct == 0),
                                    stop=(ct == 7),
                                )
                        for dc in range(2):
                            nc.scalar.copy(v_sb[:, tt, dc * 512:(dc + 1) * 512], pv[dc])

                # A2: K^T[d, tok] = sum_ct wk[ct, d].T @ xt[ct, tok]  (+ bk)
                with tc.tile_pool(name="psK", bufs=2, space="PSUM") as psK:
                    for dt in range(8):
                        pk = [psK.tile([P, 512], F32, tag=f"pk{c}", name=f"pk{c}") for c in range(4)]
                        for ct in range(8):
                            for c in range(4):
                                nc.tensor.matmul(
                                    pk[c],
                                    wk_sb[:, ct, dt * P:(dt + 1) * P],
                                    xt_sb[:, ct, c * 512:(c + 1) * 512],
                                    start=(ct == 0),
                                    stop=(ct == 7),
                                )
                        for c in range(4):
                            nc.scalar.activation(
                                kt_sb[:, dt, c * 512:(c + 1) * 512], pk[c],
                                AF.Identity, bias=bkm_sb[:, dt:dt + 1],
                            )

            # ---------------- Phase A3: Q^T for this core's 1024 rows ------
            with tc.tile_pool(name="a3_in", bufs=1) as a3_in:
                xtq_sb = a3_in.tile([P, 8, D], BF16)
                wq_sb = a3_in.tile([P, 8, D], BF16)
                for i in range(8):
                    nc.sync.dma_start(out=xtq_sb[:, i, :], in_=xtq[i * P:(i + 1) * P, :])
                    nc.sync.dma_start(out=wq_sb[:, i, :], in_=wq[i * P:(i + 1) * P, :])
                with tc.tile_pool(name="psQ", bufs=2, space="PSUM") as psQ:
                    for dt in range(8):
                        pq = [psQ.tile([P, 512], F32, tag=f"pq{c}", name=f"pq{c}") for c in range(2)]
                        for ct in range(8):
                            for c in range(2):
                                nc.tensor.matmul(
                                    pq[c],
                                    wq_sb[:, ct, dt * P:(dt + 1) * P],
                                    xtq_sb[:, ct, c * 512:(c + 1) * 512],
                                    start=(ct == 0),
                                    stop=(ct == 7),
                                )
                        for c in range(2):
                            nc.scalar.activation(
                                qt_sb[:, dt, c * 512:(c + 1) * 512], pq[c],
                                AF.Identity, bias=bqm_sb[:, dt:dt + 1],
                            )

            # ---------------- Phase B: attention per q-tile ----------------
            with tc.tile_pool(name="bwork", bufs=2) as bwork, \
                 tc.tile_pool(name="psS", bufs=1, space="PSUM") as psS, \
                 tc.tile_pool(name="psT", bufs=2, space="PSUM") as psT, \
                 tc.tile_pool(name="psO", bufs=1, space="PSUM") as psO:
                for t in range(NQT):
                    nk = 256 * (t + 1)
                    nkc = (nk + 511) // 512  # 512-chunks (last may be 256)

                    ps = psS.tile([P, 2048], F32, tag="s")
                    # S[q, k] += Q^T[dt,qtile].T @ K^T[dt, kchunk]
                    for dt in range(8):
                        for c in range(nkc):
                            w = min(512, nk - c * 512)
                            nc.tensor.matmul(
                                ps[:, c * 512:c * 512 + w],
                                qt_sb[:, dt, t * P:(t + 1) * P],
                                kt_sb[:, dt, c * 512:c * 512 + w],
                                start=(dt == 0),
                                stop=(dt == 7),
                            )
                    # causal mask on the trailing 256 cols
                    nc.vector.tensor_add(ps[:, nk - 256:nk], ps[:, nk - 256:nk],
                                         mask_sb)
                    # P = exp(S/sqrt(D)), row sums on the fly
                    p_sb = bwork.tile([P, 2048], BF16, tag="p")
                    rsum = bwork.tile([P, 1], F32, tag="rsum")
                    nc.scalar.activation(p_sb[:, :nk], ps[:, :nk], AF.Exp,
                                         scale=SCALE, accum_out=rsum)
                    rinv = bwork.tile([P, 1], F32, tag="rinv")
                    nc.vector.reciprocal(rinv, rsum)
                    nc.vector.tensor_scalar_mul(p_sb[:, :nk], p_sb[:, :nk], rinv)

                    # O = P V + 1 (x) bv
                    po = [psO.tile([P, 512], F32, tag=f"po{dc}", name=f"po{dc}") for dc in range(2)]
                    for kt in range(nk // P):
                        ptp = psT.tile([P, P], BF16, tag="ptp")
                        nc.tensor.transpose(ptp, p_sb[:, kt * P:(kt + 1) * P], ident)
                        pt_sb = bwork.tile([P, P], BF16, tag="pt")
                        nc.vector.tensor_copy(pt_sb, ptp)
                        for dc in range(2):
                            nc.tensor.matmul(
                                po[dc],
                                pt_sb,
                                v_sb[:, kt, dc * 512:(dc + 1) * 512],
                                start=(kt == 0),
                                stop=False,
                            )
                    for dc in range(2):
                        nc.tensor.matmul(
                            po[dc],
                            ones_sb,
                            bvr_sb[:, dc * 512:(dc + 1) * 512],
                            start=False,
                            stop=True,
                        )
                    o_sb = bwork.tile([P, D], F32, tag="o")
                    for dc in range(2):
                        nc.scalar.copy(o_sb[:, dc * 512:(dc + 1) * 512], po[dc])
                    nc.sync.dma_start(out=out[t * P:(t + 1) * P, :], in_=o_sb)

    nc.finalize()
    return nc


def _prep_inputs(x, wq, bq, wk, bk, wv, bv):
    bf = ml_dtypes.bfloat16
    wq_b = np.ascontiguousarray(wq, np.float32).astype(bf)
    wk_b = np.ascontiguousarray(wk, np.float32).astype(bf)
    wv_b = np.ascontiguousarray(wv, np.float32).astype(bf)
    bqm = np.ascontiguousarray(np.asarray(bq, np.float32).reshape(8, P).T)
    bkm = np.ascontiguousarray(np.asarray(bk, np.float32).reshape(8, P).T)
    bvr = np.asarray(bv, np.float32).reshape(1, D).astype(bf)

    i = np.arange(P)[:, None]
    j = np.arange(256)[None, :]
    masks = [np.where(j <= i + P * h, 0.0, NEG).astype(np.float32)
             for h in range(2)]

    in_maps = []
    for core in range(8):
        b, h = core // 2, core % 2
        xT = np.ascontiguousarray(np.asarray(x[b], np.float32).T).astype(bf)
        qcols = (np.arange(8)[:, None] * 2 + h) * P + np.arange(P)[None, :]
        xTq = np.ascontiguousarray(xT[:, qcols.ravel()])
        in_maps.append({
            "xt": xT, "xtq": xTq, "wq": wq_b, "wk": wk_b, "wv": wv_b,
            "bqm": bqm, "bkm": bkm, "bvr": bvr, "mask": masks[h],
        })
    return in_maps


def kernel(x, wq, bq, wk, bk, wv, bv, _trace=False, _trace_kwargs=None):
    if "nc" not in _CACHED:
        _CACHED["nc"] = build_nc()
    nc = _CACHED["nc"]
    in_maps = _prep_inputs(x, wq, bq, wk, bk, wv, bv)
    kw = {}
    if _trace:
        kw = dict(trace=True, **(_trace_kwargs or {}))
    res = run_bass_kernel_spmd(nc, in_maps, list(range(8)), **kw)
    out = np.empty((B, L, D), np.float32)
    for core in range(8):
        b, h = core // 2, core % 2
        o = np.asarray(res.results[core]["out"], np.float32)
        out[b].reshape(16, P, D)[h::2] = o.reshape(NQT, P, D)
    if _trace:
        _CACHED["last_results"] = res
    return out


# revision 6
# speedup vs baseline: 1.1580x; 1.1580x over previous
"""MiniCausalAttention on 8 NeuronCores (Trainium2, Bass/Tile).

Problem: x[4,2048,1024] fp32; q/k/v = x@w+b; causal softmax(q k^T/sqrt(D)) @ v.

Sharding: 8 cores = (batch b in 0..3) x (half h in 0..1). Core (b,h) handles
query tiles t' = 2t+h for t in 0..7 (interleaved 128-row tiles), so every
core sees the SAME set of causal key-extents nk(t) = 256*(t+1) -> one SPMD
program, perfectly balanced. Each core computes K^T and V for its batch's
full sequence (replicated within the pair), Q^T for its own 1024 rows.

All matmuls run in bf16 (full PE rate); PSUM accumulation is fp32; softmax
statistics fp32. Layouts: x^T/K^T/Q^T are [d_model partition, token free] so
scores S[q,k] = (Q^T)^T K^T comes out query-major; V is token-major so
O = P V after a PE transpose of each 128x128 P tile.
"""

import sys

if "/opt/trn_rl_repo" not in sys.path:
    sys.path.insert(0, "/opt/trn_rl_repo")

import numpy as np
import ml_dtypes

import concourse.bass as bass  # noqa: F401  (registers AP machinery)
import concourse.tile as tile
from concourse import bacc, mybir
from concourse.bass_utils import run_bass_kernel_spmd
from concourse.masks import make_identity

BF16 = mybir.dt.bfloat16
F32 = mybir.dt.float32
AF = mybir.ActivationFunctionType

B, L, D = 4, 2048, 1024
P = 128
NQT = 8          # q-tiles per core, 128 rows each
SCALE = 1.0 / 32.0   # 1/sqrt(D)
NEG = -1.0e30

_CACHED = {}


def build_nc():
    nc = bacc.Bacc(None, target_bir_lowering=False)

    xt = nc.declare_dram_parameter("xt", [D, L], BF16, isOutput=False)
    xtq = nc.declare_dram_parameter("xtq", [D, D], BF16, isOutput=False)
    wq = nc.declare_dram_parameter("wq", [D, D], BF16, isOutput=False)
    wk = nc.declare_dram_parameter("wk", [D, D], BF16, isOutput=False)
    wv = nc.declare_dram_parameter("wv", [D, D], BF16, isOutput=False)
    bqm = nc.declare_dram_parameter("bqm", [P, 8], F32, isOutput=False)
    bkm = nc.declare_dram_parameter("bkm", [P, 8], F32, isOutput=False)
    bvr = nc.declare_dram_parameter("bvr", [1, D], BF16, isOutput=False)
    mask = nc.declare_dram_parameter("mask", [P, 256], F32, isOutput=False)
    out = nc.declare_dram_parameter("out", [D, D], F32, isOutput=True)

    with tile.TileContext(nc) as tc:
        with tc.tile_pool(name="persist", bufs=1) as persist:
            kt_sb = persist.tile([P, 8, L], BF16)    # K^T: [d-part, dt, token]
            v_sb = persist.tile([P, 16, D], BF16)    # V: [tok-part, tt, d]
            qt_sb = persist.tile([P, 8, D], BF16)    # Q^T: [d-part, dt, qcol]
            bqm_sb = persist.tile([P, 8], F32)
            bkm_sb = persist.tile([P, 8], F32)
            bvr_sb = persist.tile([1, D], BF16)
            mask_sb = persist.tile([P, 256], F32)
            ident = persist.tile([P, P], BF16)
            ones_sb = persist.tile([1, P], BF16)

            bvb_sb = persist.tile([P, D], F32)   # broadcast bias: ones (x) bv

            nc.sync.dma_start(out=bqm_sb, in_=bqm[:, :])
            nc.sync.dma_start(out=bkm_sb, in_=bkm[:, :])
            nc.sync.dma_start(out=bvr_sb, in_=bvr[:, :])
            nc.sync.dma_start(out=mask_sb, in_=mask[:, :])
            make_identity(nc, ident)
            nc.vector.memset(ones_sb, 1.0)

            # bvb = 1 (x) bv, built once via a K=1 matmul
            with tc.tile_pool(name="psB", bufs=1, space="PSUM") as psB:
                for dc in range(2):
                    pb = psB.tile([P, 512], F32, tag="pb", name="pb")
                    nc.tensor.matmul(pb, ones_sb, bvr_sb[:, dc * 512:(dc + 1) * 512],
                                     start=True, stop=True)
                    nc.scalar.copy(bvb_sb[:, dc * 512:(dc + 1) * 512], pb)

            # ---------------- Phase A1/A2: V and K^T over full sequence ----
            # xt arrives in 4 token-chunk tiles (tc-major DMA order) so V
            # projection for early tokens starts after ~2.25 MB instead of
            # waiting for the full 4 MB.
            with tc.tile_pool(name="a_in", bufs=1) as a_in:
                wv_sb = a_in.tile([P, 8, D], BF16)
                xt_c = [a_in.tile([P, 8, 512], BF16, tag=f"xtc{c}", name=f"xtc{c}")
                        for c in range(4)]
                wk_sb = a_in.tile([P, 8, D], BF16)
                for i in range(8):
                    nc.sync.dma_start(out=wv_sb[:, i, :], in_=wv[i * P:(i + 1) * P, :])
                for c in range(4):
                    for i in range(8):
                        nc.sync.dma_start(
                            out=xt_c[c][:, i, :],
                            in_=xt[i * P:(i + 1) * P, c * 512:(c + 1) * 512])
                for i in range(8):
                    nc.sync.dma_start(out=wk_sb[:, i, :], in_=wk[i * P:(i + 1) * P, :])

                # A1: V[tok, d] = sum_ct xt[ct, tok].T @ wv[ct, d]
                with tc.tile_pool(name="psV", bufs=3, space="PSUM") as psV:
                    for tt in range(16):
                        xtile = xt_c[tt // 4]
                        toff = (tt % 4) * P
                        pv = [psV.tile([P, 512], F32, tag=f"pv{dc}", name=f"pv{dc}") for dc in range(2)]
                        for ct in range(8):
                            for dc in range(2):
                                nc.tensor.matmul(
                                    pv[dc],
                                    xtile[:, ct, toff:toff + P],
                                    wv_sb[:, ct, dc * 512:(dc + 1) * 512],
                                    start=(ct == 0),
                                    stop=(ct == 7),
                                )
                        for dc in range(2):
                            nc.scalar.copy(v_sb[:, tt, dc * 512:(dc + 1) * 512], pv[dc])

                # A2: K^T[d, tok] = sum_ct wk[ct, d].T @ xt[ct, tok]  (+ bk)
                with tc.tile_pool(name="psK", bufs=2, space="PSUM") as psK:
                    for dt in range(8):
                        pk = [psK.tile([P, 512], F32, tag=f"pk{c}", name=f"pk{c}") for c in range(4)]
                        for ct in range(8):
                            for c in range(4):
                                nc.tensor.matmul(
                                    pk[c],
                                    wk_sb[:, ct, dt * P:(dt + 1) * P],
                                    xt_c[c][:, ct, :],
                                    start=(ct == 0),
                                    stop=(ct == 7),
                                )
                        for c in range(4):
                            nc.scalar.activation(
                                kt_sb[:, dt, c * 512:(c + 1) * 512], pk[c],
                                AF.Identity, bias=bkm_sb[:, dt:dt + 1],
                            )

            # ---------------- Phase A3: Q^T for this core's 1024 rows ------
            with tc.tile_pool(name="a3_in", bufs=1) as a3_in:
                xtq_sb = a3_in.tile([P, 8, D], BF16)
                wq_sb = a3_in.tile([P, 8, D], BF16)
                for i in range(8):
                    nc.sync.dma_start(out=xtq_sb[:, i, :], in_=xtq[i * P:(i + 1) * P, :])
                    nc.sync.dma_start(out=wq_sb[:, i, :], in_=wq[i * P:(i + 1) * P, :])
                with tc.tile_pool(name="psQ", bufs=2, space="PSUM") as psQ:
                    for dt in range(8):
                        pq = [psQ.tile([P, 512], F32, tag=f"pq{c}", name=f"pq{c}") for c in range(2)]
                        for ct in range(8):
                            for c in range(2):
                                nc.tensor.matmul(
                                    pq[c],
                                    wq_sb[:, ct, dt * P:(dt + 1) * P],
                                    xtq_sb[:, ct, c * 512:(c + 1) * 512],
                                    start=(ct == 0),
                                    stop=(ct == 7),
                                )
                        for c in range(2):
                            nc.scalar.activation(
                                qt_sb[:, dt, c * 512:(c + 1) * 512], pq[c],
                                AF.Identity, bias=bqm_sb[:, dt:dt + 1],
                            )

            # ---------------- Phase B: attention per q-tile ----------------
            # Key-chunk-outer S so each 512-wide chunk finishes early; exp
            # runs per chunk (pipelined under the next chunk's matmuls);
            # P stays unnormalized through P@V and O is scaled by 1/rowsum
            # at copy-out, then bvb (= 1 (x) bv) is added.
            with tc.tile_pool(name="bwork", bufs=2) as bwork, \
                 tc.tile_pool(name="psS", bufs=4, space="PSUM") as psS, \
                 tc.tile_pool(name="psT", bufs=2, space="PSUM") as psT, \
                 tc.tile_pool(name="psO", bufs=1, space="PSUM") as psO:
                for t in range(NQT):
                    nk = 256 * (t + 1)
                    nkc = (nk + 511) // 512  # 512-chunks (last may be 256)

                    p_sb = bwork.tile([P, 2048], BF16, tag="p")
                    rsum = bwork.tile([P, 4], F32, tag="rsum")
                    po = [psO.tile([P, 512], F32, tag=f"po{dc}", name=f"po{dc}") for dc in range(2)]

                    for c in range(nkc):
                        w = min(512, nk - c * 512)
                        ps = psS.tile([P, 512], F32, tag="s")
                        for dt in range(8):
                            nc.tensor.matmul(
                                ps[:, :w],
                                qt_sb[:, dt, t * P:(t + 1) * P],
                                kt_sb[:, dt, c * 512:c * 512 + w],
                                start=(dt == 0),
                                stop=(dt == 7),
                            )
                        if c == nkc - 1:
                            # causal mask on the trailing 256 cols
                            nc.vector.tensor_add(ps[:, w - 256:w],
                                                 ps[:, w - 256:w], mask_sb)
                        nc.scalar.activation(
                            p_sb[:, c * 512:c * 512 + w], ps[:, :w], AF.Exp,
                            scale=SCALE, accum_out=rsum[:, c:c + 1])
                        # P@V for this chunk (unnormalized)
                        for kt in range(c * 4, c * 4 + w // P):
                            ptp = psT.tile([P, P], BF16, tag="ptp")
                            nc.tensor.transpose(
                                ptp, p_sb[:, kt * P:(kt + 1) * P], ident)
                            pt_sb = bwork.tile([P, P], BF16, tag="pt")
                            nc.vector.tensor_copy(pt_sb, ptp)
                            for dc in range(2):
                                nc.tensor.matmul(
                                    po[dc],
                                    pt_sb,
                                    v_sb[:, kt, dc * 512:(dc + 1) * 512],
                                    start=(kt == 0),
                                    stop=(kt == nk // P - 1),
                                )

                    rinv = bwork.tile([P, 1], F32, tag="rinv")
                    rtot = bwork.tile([P, 1], F32, tag="rtot")
                    nc.vector.reduce_sum(rtot, rsum[:, :nkc], axis=mybir.AxisListType.X)
                    nc.vector.reciprocal(rinv, rtot)
                    o_sb = bwork.tile([P, D], F32, tag="o")
                    for dc in range(2):
                        sl = slice(dc * 512, (dc + 1) * 512)
                        nc.scalar.activation(o_sb[:, sl], po[dc], AF.Copy,
                                             scale=rinv)
                        nc.vector.tensor_add(o_sb[:, sl], o_sb[:, sl],
                                             bvb_sb[:, sl])
                    nc.sync.dma_start(out=out[t * P:(t + 1) * P, :], in_=o_sb)

    nc.finalize()
    return nc


def _prep_inputs(x, wq, bq, wk, bk, wv, bv):
    bf = ml_dtypes.bfloat16
    wq_b = np.ascontiguousarray(wq, np.float32).astype(bf)
    wk_b = np.ascontiguousarray(wk, np.float32).astype(bf)
    wv_b = np.ascontiguousarray(wv, np.float32).astype(bf)
    bqm = np.ascontiguousarray(np.asarray(bq, np.float32).reshape(8, P).T)
    bkm = np.ascontiguousarray(np.asarray(bk, np.float32).reshape(8, P).T)
    bvr = np.asarray(bv, np.float32).reshape(1, D).astype(bf)

    i = np.arange(P)[:, None]
    j = np.arange(256)[None, :]
    masks = [np.where(j <= i + P * h, 0.0, NEG).astype(np.float32)
             for h in range(2)]

    in_maps = []
    for core in range(8):
        b, h = core // 2, core % 2
        xT = np.ascontiguousarray(np.asarray(x[b], np.float32).T).astype(bf)
        qcols = (np.arange(8)[:, None] * 2 + h) * P + np.arange(P)[None, :]
        xTq = np.ascontiguousarray(xT[:, qcols.ravel()])
        in_maps.append({
            "xt": xT, "xtq": xTq, "wq": wq_b, "wk": wk_b, "wv": wv_b,
            "bqm": bqm, "bkm": bkm, "bvr": bvr, "mask": masks[h],
        })
    return in_maps


def kernel(x, wq, bq, wk, bk, wv, bv, _trace=False, _trace_kwargs=None):
    if "nc" not in _CACHED:
        _CACHED["nc"] = build_nc()
    nc = _CACHED["nc"]
    in_maps = _prep_inputs(x, wq, bq, wk, bk, wv, bv)
    kw = {}
    if _trace:
        kw = dict(trace=True, **(_trace_kwargs or {}))
    res = run_bass_kernel_spmd(nc, in_maps, list(range(8)), **kw)
    out = np.empty((B, L, D), np.float32)
    for core in range(8):
        b, h = core // 2, core % 2
        o = np.asarray(res.results[core]["out"], np.float32)
        out[b].reshape(16, P, D)[h::2] = o.reshape(NQT, P, D)
    if _trace:
        _CACHED["last_results"] = res
    return out
